# revision 1
# baseline (speedup 1.0000x reference)
"""Trainium2 Bass kernel: single-channel 15x15 cross-correlation (pad=1,
stride=1) of a 4096x4096 fp32 image, + scalar bias.

Strategy
--------
Output is 4084x4084 (padded here to a 4104x4096 grid).  The conv is computed
as banded ("Toeplitz") matmuls on the tensor engine: for each of the 15
kernel columns dj, a stationary band matrix A_dj[k, m] = W[k-m, dj]
(k in [0,128) input rows, m in [0,114) output rows) multiplies a
column-shifted slice of the input tile, accumulating all 15 dj into one
PSUM tile [114, 512].  All operands are float32r (fp32 bits, fp22 multiply
inside the PE) which runs at full PE rate for free dim >= 256.

Work is tiled as 36 row-blocks (114 output rows each) x 8 column chunks
(512 cols) = 288 tiles, split evenly across 8 NeuronCores: each core owns
4 consecutive row-blocks (32 tiles) plus half of one shared row-block
(4 tiles) = 36 tiles = 540 matmuls.  Halos are included in each core's
host-side input slices, so no collectives are needed.

Input DMA is chunk-granular ([128, 526] per tile) so the tensor engine
starts ~3us into the kernel and never stalls on loads.
"""

import os

import numpy as np

KH = KW = 15
PAD = 1
H = W = 4096
OUT = H + 2 * PAD - KH + 1  # 4084
NCORES = 8
BLK_M = 114  # output rows per row-block (128 - (KH - 1))
NBLK = 36  # global row-blocks: 36*114 = 4104 >= 4084
OWN_BLKS = 4  # row-blocks fully owned by each core (32 total)
SH_CHUNKS = 4  # chunks of the shared row-block each core handles
CHUNK = 512  # output cols per PSUM tile
NCHUNK = 8  # 8*512 = 4096 >= 4084
XW = CHUNK + KW - 1  # 526: input cols per chunk tile
OWN_ROWS = OWN_BLKS * BLK_M  # 456 output rows owned per core
XROWS_OWN = OWN_ROWS + KH - 1  # 470 input rows per core (own part)
XCOLS = NCHUNK * CHUNK + KW - 1  # 4110 input cols
SH_XCOLS = SH_CHUNKS * CHUNK + KW - 1  # 2062 input cols (shared part)
XBIG_ROWS = NBLK * BLK_M + KH - 1  # 4118 padded input rows
SH_BLK0 = 32  # first shared row-block index

LAST_RESULT = None  # BassKernelResults of the most recent run (for test.py)


def _patch_drain():
    """walrus's CTRL_NO instruction struct holds very few semaphore waits;
    Tile's kernel-tail drain aggregates one wait per logical processor and
    overflows it.  Spread the waits across 1-wait-per-nop SP instructions."""
    import concourse.mybir as mybir
    import concourse.tile as tile
    from concourse.vector_clock import ScopedClock

    def _split_drain_and_barrier(self, tick_clock, wait_clock):
        nc = self.nc
        probe = nc.sync.nop(nofuse=True)
        wait_clock.add_sem_waits(
            probe.ins, ScopedClock({None: tick_clock.global_clock})
        )
        si = probe.ins.sync_info
        if si is not None and len(si.on_wait) > 1:
            waits = list(si.on_wait)
            probe.ins.sync_info = mybir.SyncInfo(
                on_wait=waits[:1], on_update=list(si.on_update)
            )
            for w in waits[1:]:
                extra = nc.sync.nop(nofuse=True)
                extra.ins.sync_info = mybir.SyncInfo(on_wait=[w], on_update=[])
        nc.sync.drain()
        # The stock exit path does barrier -> semaphore cleanup -> barrier
        # (~8us).  This NEFF executes once per load, so leftover semaphore
        # values don't matter: skip the cleanup, keep only the drain (which
        # carries the waits that guarantee all DMAs have landed).
        assert self.sems is not None
        popped = nc._tile_sem_poison_stack.pop()
        assert popped is self._sem_poison

    tile.TileContext._drain_and_barrier = _split_drain_and_barrier


def _split_multi_waits(nc):
    """This compiler's TPB instruction structs hold only one sync-wait slot
    (walrus setupSyncWait rejects more).  Tile sometimes assigns 2+ waits
    (DMA completion + slot release) to one instruction; split the excess onto
    same-engine nops inserted immediately before it."""
    import concourse.mybir as mybir

    for fn in nc.m.functions:
        for bb in fn.blocks:
            insts = list(bb.instructions)
            out = []
            changed = False
            for inst in insts:
                si = inst.sync_info
                if (
                    not isinstance(inst, mybir.InstNoOp)
                    and si is not None
                    and len(si.on_wait) > 1
                ):
                    waits = list(si.on_wait)
                    for w in waits[:-1]:
                        nop = mybir.InstNoOp(
                            name=nc.get_next_instruction_name(),
                            engine=inst.engine,
                            bass_nofuse=True,
                            sync_info=mybir.SyncInfo(on_wait=[w], on_update=[]),
                        )
                        nc.register_instruction(nop)
                        out.append(nop)
                    inst.sync_info = mybir.SyncInfo(
                        on_wait=[waits[-1]], on_update=list(si.on_update)
                    )
                    changed = True
                out.append(inst)
            if changed:
                bb.instructions = out


def _make_bands(weight):
    """bands[k, dj*BLK_M + m] = W[k-m, dj] for k-m in [0, KH)."""
    A = np.zeros((128, KW, BLK_M), np.float32)
    idx = np.arange(BLK_M)
    for dj in range(KW):
        for di in range(KH):
            A[idx + di, dj, idx] = weight[di, dj]
    return np.ascontiguousarray(A.reshape(128, KW * BLK_M))


def _build_program(bias_val):
    import concourse.bass as bass
    import concourse.mybir as mybir
    import concourse.tile as tile

    _patch_drain()
    f32r = mybir.dt.float32r
    f32 = mybir.dt.float32

    nc = bass.Bass()
    x_own = nc.declare_dram_parameter("x_own", [XROWS_OWN, XCOLS], f32r, isOutput=False)
    x_sh = nc.declare_dram_parameter("x_sh", [128, SH_XCOLS], f32r, isOutput=False)
    bands = nc.declare_dram_parameter("bands", [128, KW * BLK_M], f32r, isOutput=False)
    out_own = nc.declare_dram_parameter(
        "out_own", [OWN_ROWS, NCHUNK * CHUNK], f32, isOutput=True
    )
    out_sh = nc.declare_dram_parameter(
        "out_sh", [BLK_M, SH_CHUNKS * CHUNK], f32, isOutput=True
    )

    with tile.TileContext(nc) as tc:
        with (
            tc.tile_pool(name="const", bufs=1) as constp,
            tc.tile_pool(name="xp", bufs=8) as xp,
            tc.tile_pool(name="psum", bufs=4, space="PSUM") as psp,
            tc.tile_pool(name="op", bufs=4) as outp,
        ):
            # 15 separate band tiles so the first matmul only waits for the
            # first 58 KB load, not the whole 875 KB bands tensor.  Bands and
            # output stores ride the Activation engine's HWDGE ring; the SP
            # ring is reserved for input tiles so the first x chunk is the
            # first transfer in its queue.
            bts = []
            for dj in range(KW):
                bt = constp.tile([128, BLK_M], f32r, tag=f"band{dj}")
                nc.scalar.dma_start(out=bt[:, :], in_=bands[:, BLK_M * dj : BLK_M * (dj + 1)])
                bts.append(bt)

            def do_tile(src, r0, c0, dst, dr0, dc0):
                """One [114, 512] output tile: 15 banded matmuls + evac."""
                xt = xp.tile([128, XW], f32r, tag="xt")
                nc.sync.dma_start(out=xt[:, :], in_=src[r0 : r0 + 128, c0 : c0 + XW])
                ps = psp.tile([BLK_M, CHUNK], f32, tag="ps")
                for dj in range(KW):
                    nc.tensor.matmul(
                        ps[:, :],
                        bts[dj][:, :],
                        xt[:, dj : dj + CHUNK],
                        start=(dj == 0),
                        stop=(dj == KW - 1),
                    )
                ot = outp.tile([BLK_M, CHUNK], f32, tag="ot")
                nc.vector.tensor_scalar_add(ot[:, :], ps[:, :], bias_val)
                nc.scalar.dma_start(
                    out=dst[dr0 : dr0 + BLK_M, dc0 : dc0 + CHUNK], in_=ot[:, :]
                )

            for b in range(OWN_BLKS):
                for q in range(NCHUNK):
                    do_tile(x_own, BLK_M * b, CHUNK * q, out_own, BLK_M * b, CHUNK * q)
            for q in range(SH_CHUNKS):
                do_tile(x_sh, 0, CHUNK * q, out_sh, 0, CHUNK * q)

    _split_multi_waits(nc)
    return nc


def kernel(x, weight, bias):
    global LAST_RESULT
    from concourse.bass_utils import run_bass_kernel_spmd

    x = np.ascontiguousarray(np.asarray(x, dtype=np.float32))
    weight = np.asarray(weight, dtype=np.float32)
    bias = np.asarray(bias, dtype=np.float32)

    # Host-side zero padding: PAD on top/left, plus enough extra rows/cols
    # that every core's fixed-size slice stays in bounds.
    xbig = np.zeros((XBIG_ROWS, XCOLS), np.float32)
    xbig[PAD : PAD + H, PAD : PAD + W] = x
    bands = _make_bands(weight)

    nc = _build_program(float(bias[0]))
    in_maps = []
    for c in range(NCORES):
        sh_blk = SH_BLK0 + c // 2
        sh_col = (SH_CHUNKS * CHUNK) * (c % 2)
        in_maps.append(
            {
                "x_own": np.ascontiguousarray(
                    xbig[OWN_ROWS * c : OWN_ROWS * c + XROWS_OWN]
                ),
                "x_sh": np.ascontiguousarray(
                    xbig[BLK_M * sh_blk : BLK_M * sh_blk + 128, sh_col : sh_col + SH_XCOLS]
                ),
                "bands": bands,
            }
        )
    res = run_bass_kernel_spmd(
        nc,
        in_maps,
        list(range(NCORES)),
        trace=bool(os.environ.get("CONV_TRACE")),
    )
    LAST_RESULT = res

    full = np.empty((NBLK * BLK_M, NCHUNK * CHUNK), np.float32)
    for c in range(NCORES):
        r = res.results[c]
        full[OWN_ROWS * c : OWN_ROWS * (c + 1)] = r["out_own"]
        sh_blk = SH_BLK0 + c // 2
        sh_col = (SH_CHUNKS * CHUNK) * (c % 2)
        full[
            BLK_M * sh_blk : BLK_M * (sh_blk + 1), sh_col : sh_col + SH_CHUNKS * CHUNK
        ] = r["out_sh"]
    return np.ascontiguousarray(full[:OUT, :OUT]).astype(np.float32)



# revision 6
# speedup vs baseline: 1.8036x; 1.8036x over previous
"""Trainium2 Bass kernel: single-channel 15x15 cross-correlation (pad=1,
stride=1) of a 4096x4096 fp32 image, + scalar bias.

Strategy
--------
fp8 (e4m3) matmuls in DoubleRow perf mode (K=256, 0.5 PE cycles per output
column) with the image columns folded 8-wide into the partition dim:
partition p = (row-in-block)*8 + col-phase, one k-tile block = 16 image rows.
DoubleRow pairs two adjacent blocks, so one matmul contracts a 32-row x
8-phase input window against a banded weight matrix whose M=128 output
partitions are 16 output rows x 8 col-phases.  A 15x15 kernel then needs only
3 column-shift matmuls (dj = 8t + s_in - s_out) per conv term.

fp8 alone is too coarse (rel err ~3e-2), so the conv is split into three fp8
convs: x_hi*w_hi + x_lo*w_hi + x_hi*w_lo with x = x_hi + x_lo, w = w_hi +
w_lo both split host-side into fp8 value + fp8 residual (dropping the
second-order term), giving ~1e-3 rel err.  9 matmuls of N=511 accumulate one
PSUM bank per strip of 16 output rows x 4088 output cols.

Each core owns 32 strips (512 output rows): 9*32 matmuls * 511 cols * 0.5
cycles at 2.4 GHz ~= 31 us of PE stream time.  Inputs load once (33 folded
blocks per array per core, halo included host-side, no collectives); outputs
store as folded bf16 and the host unfolds, crops, and adds the bias.
"""

import os

import numpy as np

KH = KW = 15
PAD = 1
H = W = 4096
OUT = H + 2 * PAD - KH + 1  # 4084
NCORES = 8
F = 8  # column fold factor
RB = 16  # image rows per fold block (one k-tile: RB*F = 128 partitions)
NSTRIP = 32  # strips (16 output rows each) per core
NBLK = NSTRIP + 1  # fold blocks per core (one halo block)
NFC = 513  # folded input cols per block (8*513 = 4104 padded cols)
NFO = 511  # folded output cols per strip (8*511 = 4088 >= 4084)
NT = 3  # column-shift matmuls per conv term
XROWS = RB * (NSTRIP * NCORES + 1)  # 4112 padded image rows
XCOLS = F * NFC  # 4104 padded image cols
DMA_BLKS = 3  # fold blocks per input DMA

LAST_RESULT = None  # BassKernelResults of the most recent run (for test.py)


def _patch_drain():
    """walrus's CTRL_NO instruction struct holds very few semaphore waits;
    Tile's kernel-tail drain aggregates one wait per logical processor and
    overflows it.  Spread the waits across 1-wait-per-nop SP instructions."""
    import concourse.mybir as mybir
    import concourse.tile as tile
    from concourse.vector_clock import ScopedClock

    def _split_drain_and_barrier(self, tick_clock, wait_clock):
        nc = self.nc
        probe = nc.sync.nop(nofuse=True)
        wait_clock.add_sem_waits(
            probe.ins, ScopedClock({None: tick_clock.global_clock})
        )
        si = probe.ins.sync_info
        if si is not None and len(si.on_wait) > 1:
            waits = list(si.on_wait)
            probe.ins.sync_info = mybir.SyncInfo(
                on_wait=waits[:1], on_update=list(si.on_update)
            )
            for w in waits[1:]:
                extra = nc.sync.nop(nofuse=True)
                extra.ins.sync_info = mybir.SyncInfo(on_wait=[w], on_update=[])
        nc.sync.drain()
        # The stock exit path does barrier -> semaphore cleanup -> barrier
        # (~8us).  This NEFF executes once per load, so leftover semaphore
        # values don't matter: skip the cleanup, keep only the drain (which
        # carries the waits that guarantee all DMAs have landed).
        assert self.sems is not None
        popped = nc._tile_sem_poison_stack.pop()
        assert popped is self._sem_poison

    tile.TileContext._drain_and_barrier = _split_drain_and_barrier


def _split_multi_waits(nc):
    """This compiler's TPB instruction structs hold only one sync-wait slot
    (walrus setupSyncWait rejects more).  Tile sometimes assigns 2+ waits
    (DMA completion + slot release) to one instruction; split the excess onto
    same-engine nops inserted immediately before it."""
    import concourse.mybir as mybir

    for fn in nc.m.functions:
        for bb in fn.blocks:
            insts = list(bb.instructions)
            out = []
            changed = False
            for inst in insts:
                si = inst.sync_info
                if (
                    not isinstance(inst, mybir.InstNoOp)
                    and si is not None
                    and len(si.on_wait) > 1
                ):
                    waits = list(si.on_wait)
                    for w in waits[:-1]:
                        nop = mybir.InstNoOp(
                            name=nc.get_next_instruction_name(),
                            engine=inst.engine,
                            bass_nofuse=True,
                            sync_info=mybir.SyncInfo(on_wait=[w], on_update=[]),
                        )
                        nc.register_instruction(nop)
                        out.append(nop)
                    inst.sync_info = mybir.SyncInfo(
                        on_wait=[waits[-1]], on_update=list(si.on_update)
                    )
                    changed = True
                out.append(inst)
            if changed:
                bb.instructions = out


def _make_bands(w):
    """B[t][p, i, m] = w[di, dj] with di = 16i + p//8 - m//8,
    dj = 8t + p%8 - m%8 (zero outside the 15x15 support)."""
    B = np.zeros((NT, 128, 2, 128), np.float32)
    p = np.arange(128)
    r_, s_in = p // 8, p % 8
    m = np.arange(128)
    m_row, s_out = m // 8, m % 8
    for t in range(NT):
        for i in range(2):
            di = (RB * i + r_)[:, None] - m_row[None, :]
            dj = F * t + s_in[:, None] - s_out[None, :]
            valid = (di >= 0) & (di < KH) & (dj >= 0) & (dj < KW)
            B[t, :, i, :][valid] = w[di[valid], dj[valid]]
    return B


def _fold(arr8):
    """[XROWS, XCOLS] fp8 -> [nblocks, 128, NFC]: block g holds image rows
    [16g, 16g+16), partition p = (row%16)*8 + (col%8), free n = col//8."""
    g = XROWS // RB
    return np.ascontiguousarray(
        arr8.reshape(g, RB, NFC, F).transpose(0, 1, 3, 2).reshape(g, 128, NFC)
    )


def _build_program(bias_val):
    import concourse.bass as bass
    import concourse.mybir as mybir
    import concourse.tile as tile

    _patch_drain()
    f8 = mybir.dt.float8e4
    f32 = mybir.dt.float32
    bf16 = mybir.dt.bfloat16
    DR = mybir.MatmulPerfMode.DoubleRow

    nc = bass.Bass()
    xh = nc.declare_dram_parameter("xh", [128, NBLK * NFC], f8, isOutput=False)
    xl = nc.declare_dram_parameter("xl", [128, NBLK * NFC], f8, isOutput=False)
    bd = nc.declare_dram_parameter("bands", [128, 6 * 2 * 128], f8, isOutput=False)
    out = nc.declare_dram_parameter("out", [NSTRIP * 128, NFO], bf16, isOutput=True)

    with tile.TileContext(nc) as tc:
        with (
            tc.tile_pool(name="const", bufs=1) as constp,
            tc.tile_pool(name="xp", bufs=1) as xp,
            tc.tile_pool(name="psum", bufs=4, space="PSUM") as psp,
            tc.tile_pool(name="op", bufs=4) as outp,
        ):
            bt = constp.tile([128, 6, 2, 128], f8, tag="bands")
            nc.scalar.dma_start(out=bt[:, :, :, :], in_=bd[:, :])

            xht = xp.tile([128, NBLK, NFC], f8, tag="xh")
            xlt = xp.tile([128, NBLK, NFC], f8, tag="xl")
            for b0 in range(0, NBLK, DMA_BLKS):
                b1 = min(b0 + DMA_BLKS, NBLK)
                nc.sync.dma_start(
                    out=xht[:, b0:b1, :], in_=xh[:, b0 * NFC : b1 * NFC]
                )
                nc.gpsimd.dma_start(
                    out=xlt[:, b0:b1, :], in_=xl[:, b0 * NFC : b1 * NFC]
                )

            # (moving tensor, band index) per conv term: x_hi*w_hi + x_lo*w_hi
            # + x_hi*w_lo; bands 0..2 hold w_hi shifts, 3..5 w_lo shifts.
            terms = ((xht, 0), (xlt, 0), (xht, 3))
            for j in range(NSTRIP):
                ps = psp.tile([128, NFO], f32, tag="ps")
                for idx, (src, sel) in enumerate(terms):
                    for t in range(NT):
                        nc.tensor.matmul(
                            ps[:, :],
                            bt[:, sel + t, :, :],
                            src[:, j : j + 2, t : t + NFO],
                            start=(idx == 0 and t == 0),
                            stop=(idx == len(terms) - 1 and t == NT - 1),
                            perf_mode=DR,
                        )
                ot = outp.tile([128, NFO], bf16, tag="ot")
                nc.vector.tensor_scalar_add(ot[:, :], ps[:, :], bias_val)
                nc.scalar.dma_start(
                    out=out[128 * j : 128 * (j + 1), :], in_=ot[:, :]
                )

    _split_multi_waits(nc)
    return nc


def kernel(x, weight, bias):
    global LAST_RESULT
    import ml_dtypes
    from concourse.bass_utils import run_bass_kernel_spmd

    e4 = ml_dtypes.float8_e4m3
    x = np.ascontiguousarray(np.asarray(x, dtype=np.float32))
    weight = np.asarray(weight, dtype=np.float32)
    bias = np.asarray(bias, dtype=np.float32)

    xpad = np.zeros((XROWS, XCOLS), np.float32)
    xpad[PAD : PAD + H, PAD : PAD + W] = x
    x_hi = xpad.astype(e4)
    x_lo = (xpad - x_hi.astype(np.float32)).astype(e4)
    xf_hi = _fold(x_hi)
    xf_lo = _fold(x_lo)

    w_hi = weight.astype(e4).astype(np.float32)
    w_lo = (weight - w_hi).astype(e4).astype(np.float32)
    bands = np.concatenate([_make_bands(w_hi), _make_bands(w_lo)], axis=0)
    bands8 = np.ascontiguousarray(
        bands.transpose(1, 0, 2, 3).reshape(128, 6 * 2 * 128).astype(e4)
    )

    nc = _build_program(float(bias[0]))
    in_maps = []
    for c in range(NCORES):
        blk0 = NSTRIP * c
        in_maps.append(
            {
                "xh": np.ascontiguousarray(
                    xf_hi[blk0 : blk0 + NBLK].transpose(1, 0, 2)
                ).reshape(128, NBLK * NFC),
                "xl": np.ascontiguousarray(
                    xf_lo[blk0 : blk0 + NBLK].transpose(1, 0, 2)
                ).reshape(128, NBLK * NFC),
                "bands": bands8,
            }
        )
    res = run_bass_kernel_spmd(
        nc,
        in_maps,
        list(range(NCORES)),
        trace=bool(os.environ.get("CONV_TRACE")),
    )
    LAST_RESULT = res

    full = np.empty((NCORES * NSTRIP * RB, NFO * F), np.float32)
    for c in range(NCORES):
        o = np.asarray(res.results[c]["out"]).astype(np.float32)
        full[512 * c : 512 * (c + 1)] = (
            o.reshape(NSTRIP, RB, F, NFO)
            .transpose(0, 1, 3, 2)
            .reshape(NSTRIP * RB, NFO * F)
        )
    return np.ascontiguousarray(full[:OUT, :OUT]).astype(np.float32)
